# revision 29
# baseline (speedup 1.0000x reference)
"""ChebyshevKAN layer on 8 Trainium2 NeuronCores.

y[b,o] = sum_{i,j} T_j(xn[b,i]) * C[i,o,j],  xn = per-row min/max normalize to [-1,1]

Strategy (per core, batch-sharded 8 ways => 1024 rows/core):
  - x host-cast to fp16; rows normalized on ACT (per-partition scale/bias),
    DMA-transposed into [i, b] layout tiles
  - degrees 1-4 run as fp8e4 DoubleRow matmuls (K=256 per MM, measured 2.0x
    PE throughput): coeffs pre-quantized on host as zero-sum-rounded
    e4m3(C_j * 4) (zero column error sums kill the coherent error term that
    the nonzero row-means of even-degree T_j would otherwise amplify); T_2 is
    additionally centered at its distribution mean -0.797, folded into the
    bias row. Degrees 5-8 stay fp16. Measured rel err 1.575e-2 (< 2e-2).
  - the global 4x coeff scale is undone (x 0.25) in the PSUM->SBUF output
    copy, so the device-side fp8 T casts are all scale-1 ACT ops
  - T chain built via Chebyshev composition rather than the plain recurrence:
    ACT Squares produce U2=2xn^2, T4=2(U2-1)^2-1, T6=2T3^2-1, U8=2T4^2
    (T_2n = 2 T_n^2 - 1), leaving DVE only the odd chain M3=-T3, T5, T7 as
    fused scalar_tensor_tensor pairs (2x 16-bit mode on packed tiles). U8 is
    used directly as the degree-8 lhsT with the -1 folded into the bias row;
    C3's sign flip (for M3) is folded into its fp8 slab.
  - bias row (degree 0 + the centering/U8 folds) precomputed on host as a
    [1, O] vector, seeded into PSUM via rank-1 matmuls
  - pass 2 runs degrees high->low so accumulators close on the cheap fp8
    degree 1 (short copy/store tail); throwaway rank-1 matmuls fill the PE's
    initial dependency-idle window so the HAM clock gate opens early (this
    alone was worth tens of us on the repeat-loop measurement)
"""

import sys

sys.path.insert(0, "/opt/trn_rl_repo")

import numpy as np
import ml_dtypes

import concourse.bass as bass  # noqa: F401  (bass must import before tile)
import concourse.tile as tile
from concourse import bacc, mybir
from concourse.bass_utils import run_bass_kernel_spmd

NCORES = 8
B_FULL = 8192
B_SH = B_FULL // NCORES  # 1024 rows per core
I_DIM = 1024
O_DIM = 1024
NJ = 9  # degrees 0..8
P = 128
NBT = B_SH // P  # 8 batch tiles per core
NIC = I_DIM // P  # 8 contraction chunks (fp16)
NDC = I_DIM // 256  # 4 contraction chunks (fp8 DoubleRow, K=256)
OT = 512  # output tile width
NOT = O_DIM // OT  # 2

FP8_DEGREES = (1, 2, 3, 4)  # DoubleRow e4m3
FP16_DEGREES = (5, 6, 7, 8)
# centering constants (distribution-level, folded into the bias path)
CENTER = {1: 0.0, 2: -0.797, 3: 0.0, 4: 0.0}
# Global output scale: all coeffs are scaled by B_SCALE on host (fp16 and fp8
# alike), the PSUM therefore holds y*B_SCALE, and the output copy multiplies
# by 1/B_SCALE (exact power of 2). This lets the fp8 T casts use scale 1.0 --
# pure dtype copies that run on the otherwise-idle Pool engine -- while C is
# quantized at the e4m3-friendly scale 4.
B_SCALE = 4.0
OUT_SCALE = 1.0 / B_SCALE

_PROGRAM_CACHE = {}


def build_program(repeat=1, ndev=NCORES, variant="full"):
    """Build + compile the per-core Bass program (cached).

    repeat>1 wraps the whole body in an on-device loop — used only for
    timing (amortizes host dispatch overhead over `repeat` kernel runs).
    variant="pe_only" replaces phases A/B with memsets (timing diagnostic
    only -- results are garbage).
    """
    if (repeat, ndev, variant) in _PROGRAM_CACHE:
        return _PROGRAM_CACHE[(repeat, ndev, variant)]

    f16 = mybir.dt.float16
    f32 = mybir.dt.float32
    f8 = mybir.dt.float8e4

    nc = bacc.Bacc("TRN2", target_bir_lowering=False, debug=False, num_devices=ndev)
    xs_ext = nc.dram_tensor("xs", [B_SH, I_DIM], f16, kind="ExternalInput")
    # fp16 slabs for degrees 5..8, laid out [jj, i, o]
    cj16_ext = nc.dram_tensor(
        "cj16", [len(FP16_DEGREES), I_DIM, O_DIM], f16, kind="ExternalInput"
    )
    # fp8 slabs for degrees 1..4, laid out [jj, p, c, s, o] with i = c*256+s*128+p
    cj8_ext = nc.dram_tensor(
        "cj8", [len(FP8_DEGREES), P, NDC, 2, O_DIM], f8, kind="ExternalInput"
    )
    bias_ext = nc.dram_tensor("bias", [1, O_DIM], f16, kind="ExternalInput")
    y_ext = nc.dram_tensor("y", [B_SH, O_DIM], f32, kind="ExternalOutput")

    import contextlib

    with tile.TileContext(nc) as tc:
        with (
            tc.tile_pool(name="tall", bufs=1) as tp,
            tc.tile_pool(name="xp", bufs=2) as xpool,
            tc.tile_pool(name="pp", bufs=2) as ppool,
            tc.tile_pool(name="sm", bufs=16) as spool,
            tc.tile_pool(name="cp", bufs=2) as cpool,
            tc.tile_pool(name="c8", bufs=2) as c8pool,
            tc.tile_pool(name="op", bufs=2) as opool,
            tc.tile_pool(name="ps", bufs=8, space="PSUM") as pspool,
            tc.For_i(0, repeat, 1) if repeat > 1 else contextlib.nullcontext(),
        ):
            # fp16 T tiles in [i,b] layout, free dims packed per (slot, bt) so
            # DVE elementwise ops get the 2x 16-bit mode. Degrees 2..4 are
            # recurrence-only in fp16 (PE reads their fp8 casts), so T5..T7
            # reuse their slots once free: 5 slots instead of 8.
            SLOT = {1: 0, 2: 1, 3: 2, 4: 3, 5: 1, 6: 2, 7: 3, 8: 4}
            T16 = tp.tile([P, 5, NBT, NIC, P], f16)
            # fp8 (T_j - c_j)*A_SCALE tiles for degrees 1..4
            T8Q = tp.tile([P, len(FP8_DEGREES), NBT, NIC, P], f8)

            ones_row = tp.tile([1, P], f16)
            nc.vector.memset(ones_row, 1.0)

            # constant bias tiles for the ACT ops
            zbias = tp.tile([P, 1], f32, name="zbias", tag="zbias")
            nc.vector.memset(zbias, 0.0)
            nrt2 = tp.tile([P, 1], f32, name="nrt2", tag="nrt2")
            nc.vector.memset(nrt2, -(2.0 ** 0.5))
            monebias = tp.tile([P, 1], f32, name="mone", tag="mone")
            nc.vector.memset(monebias, -1.0)
            # j2's fp8 cast reads U2 = T2 + 1, so its bias is -1 - c2
            cast_bias = {2: tp.tile([P, 1], f32, name="castb2", tag="castb2")}
            nc.vector.memset(cast_bias[2], -1.0 - CENTER[2])

            # host-precomputed bias row: seeds every PSUM accumulator
            bias_sb = tp.tile([1, O_DIM], f16)
            nc.sync.dma_start(out=bias_sb, in_=bias_ext[:, :])

            if variant == "pe_only":
                nc.vector.memset(T16, 0.001)
                nc.vector.memset(T8Q, 0.001)
            # ---- Phase A: normalize + transpose, per batch tile ----
            # x arrives fp16 (host-cast): half the DMA bytes. Loads ride the
            # SP queue so they land ahead of the coeff slabs on the DMA engines.
            for bt in range(NBT) if variant != "pe_only" else []:
                x_sb = xpool.tile([P, I_DIM], f16)
                nc.sync.dma_start(out=x_sb, in_=xs_ext[bt * P : (bt + 1) * P, :])
                mx = spool.tile([P, 1], f32)
                mn = spool.tile([P, 1], f32)
                nc.vector.tensor_reduce(
                    out=mx, in_=x_sb, op=mybir.AluOpType.max, axis=mybir.AxisListType.X
                )
                nc.vector.tensor_reduce(
                    out=mn, in_=x_sb, op=mybir.AluOpType.min, axis=mybir.AxisListType.X
                )
                st2 = spool.tile([P, 2], f32)
                s = st2[:, 0:1]
                t = st2[:, 1:2]
                rng = spool.tile([P, 1], f32)
                nc.vector.tensor_sub(out=rng, in0=mx, in1=mn)
                nc.vector.reciprocal(out=s, in_=rng)
                nc.vector.tensor_scalar_mul(s, s, 2.0)
                # t = (mn * -1) * s - 1
                nc.vector.scalar_tensor_tensor(
                    out=t, in0=mn, scalar=-1.0, in1=s,
                    op0=mybir.AluOpType.mult, op1=mybir.AluOpType.mult,
                )
                nc.vector.tensor_scalar_add(t, t, -1.0)

                xt16 = xpool.tile([P, I_DIM], f16)
                nc.scalar.activation(
                    out=xt16, in_=x_sb,
                    func=mybir.ActivationFunctionType.Identity,
                    bias=t, scale=s,
                )
                # T_1 = xn, transposed into [i, b] tiles
                nc.scalar.dma_start_transpose(out=T16[:, SLOT[1], bt, :, :], in_=xt16)
                # fp8 cast of T_1 on ACT (scale 1.0 thanks to the global
                # output scale)
                nc.scalar.activation(
                    out=T8Q[:, 0, bt, :, :], in_=T16[:, SLOT[1], bt, :, :],
                    func=mybir.ActivationFunctionType.Identity,
                    bias=zbias, scale=1.0,
                )
                # wave 2 inline: U2 = 2*xn^2 (ACT Square) + j2 fp8 cast
                u2 = T16[:, SLOT[2], bt, :, :]
                nc.scalar.activation(out=u2, in_=T16[:, SLOT[1], bt, :, :],
                                     func=mybir.ActivationFunctionType.Square,
                                     bias=zbias, scale=2.0 ** 0.5)
                nc.scalar.activation(
                    out=T8Q[:, 1, bt, :, :], in_=u2,
                    func=mybir.ActivationFunctionType.Identity,
                    bias=cast_bias[2], scale=1.0,
                )

            # ---- Phase B: Chebyshev chain via composition identities ----
            # ACT computes the even degrees as Squares (T_2n = 2*T_n^2 - 1),
            # DVE only handles the odd chain + two fixups:
            #   slot1: U2 = 2*xn^2            (ACT Square of T1; = T2 + 1)
            #   slot2: M3 = 3*T1 - 2*T1*U2    (DVE 2 STTs; = -T3, C3 sign
            #                                  flipped on host)
            #   slot3: T4 = 2*(U2-1)^2 - 1    (ACT Square of U2 + DVE -1 fixup)
            #   slot1: T5 = 2*T1*T4 + M3      (DVE 2 STTs, overwrites U2)
            #   slot2: T6 = 2*M3^2 - 1        (ACT Square of M3 + DVE fixup)
            #   slot3: T7 = 2*T1*T6 - T5      (DVE 2 STTs, overwrites T4)
            #   slot4: U8 = 2*T4^2            (ACT Square; = T8 + 1, the -1 is
            #                                  folded into the host bias row)
            # fp8 casts (ACT): j1 from T1, j2 = U2 - (1 + c2), j3 = M3,
            # j4 from the U4 scratch with bias -1.
            SQ = mybir.ActivationFunctionType.Square
            RT2 = 2.0 ** 0.5
            if variant != "pe_only":
                t1_of = lambda bt: T16[:, SLOT[1], bt, :, :]
                # wave 3: M3 = (T1*3) - (2*U2)*T1 on DVE + j3 cast
                for bt in range(NBT):
                    u2 = T16[:, SLOT[2], bt, :, :]
                    m3 = T16[:, SLOT[3], bt, :, :]
                    prod = ppool.tile([P, NIC, P], f16, name="p3", tag="p3")
                    nc.vector.scalar_tensor_tensor(
                        out=prod, in0=u2, scalar=2.0, in1=t1_of(bt),
                        op0=mybir.AluOpType.mult, op1=mybir.AluOpType.mult,
                    )
                    nc.vector.scalar_tensor_tensor(
                        out=m3, in0=t1_of(bt), scalar=3.0, in1=prod,
                        op0=mybir.AluOpType.mult, op1=mybir.AluOpType.subtract,
                    )
                    nc.scalar.activation(
                        out=T8Q[:, 2, bt, :, :], in_=m3,
                        func=mybir.ActivationFunctionType.Identity,
                        bias=zbias, scale=1.0,
                    )
                # wave 4: U4 = 2*(U2-1)^2 scratch (ACT), j4 cast = U4 - 1
                # (ACT), T4 = U4 - 1 fp16 fixup (DVE)
                for bt in range(NBT):
                    u2 = T16[:, SLOT[2], bt, :, :]
                    t4 = T16[:, SLOT[4], bt, :, :]
                    u4 = ppool.tile([P, NIC, P], f16, name="u4", tag="u4")
                    nc.scalar.activation(out=u4, in_=u2, func=SQ,
                                         bias=nrt2, scale=RT2)
                    nc.scalar.activation(
                        out=T8Q[:, 3, bt, :, :], in_=u4,
                        func=mybir.ActivationFunctionType.Identity,
                        bias=monebias, scale=1.0,
                    )
                    nc.vector.tensor_scalar_add(t4, u4, -1.0)
                # wave 5: T5 = (T4*2)*T1 + M3 (DVE), overwrites U2's slot
                for bt in range(NBT):
                    t4 = T16[:, SLOT[4], bt, :, :]
                    m3 = T16[:, SLOT[3], bt, :, :]
                    t5 = T16[:, SLOT[5], bt, :, :]
                    prod = ppool.tile([P, NIC, P], f16, name="p5", tag="p5")
                    nc.vector.scalar_tensor_tensor(
                        out=prod, in0=t4, scalar=2.0, in1=t1_of(bt),
                        op0=mybir.AluOpType.mult, op1=mybir.AluOpType.mult,
                    )
                    nc.vector.scalar_tensor_tensor(
                        out=t5, in0=prod, scalar=1.0, in1=m3,
                        op0=mybir.AluOpType.mult, op1=mybir.AluOpType.add,
                    )
                # wave 6: U6 = 2*M3^2 (ACT scratch), T6 = U6 - 1 (DVE),
                # overwrites M3's slot
                for bt in range(NBT):
                    m3 = T16[:, SLOT[3], bt, :, :]
                    u6 = ppool.tile([P, NIC, P], f16, name="u6", tag="u6")
                    nc.scalar.activation(out=u6, in_=m3, func=SQ,
                                         bias=zbias, scale=RT2)
                    nc.vector.tensor_scalar_add(T16[:, SLOT[6], bt, :, :], u6, -1.0)
                # wave 8a: U8 = 2*T4^2 (ACT) - early, T4 is ready
                for bt in range(NBT):
                    t4 = T16[:, SLOT[4], bt, :, :]
                    u8 = T16[:, SLOT[8], bt, :, :]
                    nc.scalar.activation(out=u8, in_=t4, func=SQ,
                                         bias=zbias, scale=RT2)
                # wave 7: T7 = (T6*2)*T1 - T5 (DVE), overwrites T4's slot
                for bt in range(NBT):
                    t6 = T16[:, SLOT[6], bt, :, :]
                    t5 = T16[:, SLOT[5], bt, :, :]
                    t7 = T16[:, SLOT[7], bt, :, :]
                    prod = ppool.tile([P, NIC, P], f16, name="p7", tag="p7")
                    nc.vector.scalar_tensor_tensor(
                        out=prod, in0=t6, scalar=2.0, in1=t1_of(bt),
                        op0=mybir.AluOpType.mult, op1=mybir.AluOpType.mult,
                    )
                    nc.vector.scalar_tensor_tensor(
                        out=t7, in0=prod, scalar=1.0, in1=t5,
                        op0=mybir.AluOpType.mult, op1=mybir.AluOpType.subtract,
                    )

            # ---- Phase C: matmuls, coeffs streamed once per output tile ----
            # ot=0 runs degrees low->high (trailing the recurrence front);
            # ot=1 runs high->low so the accumulators close on the cheap fp8
            # degree 1 and the copy/store tail is short.
            def emit_degree(j, ot, psums, closing):
                if j in FP8_DEGREES:
                    jj = FP8_DEGREES.index(j)
                    c8_sb = c8pool.tile(
                        [P, NDC * 2, OT], f8, name=f"c8_{ot}_{j}", tag="c8_sb"
                    )
                    nc.sync.dma_start(
                        out=c8_sb,
                        in_=cj8_ext[jj, :, :, :, ot * OT : (ot + 1) * OT].rearrange(
                            "p c s o -> p (c s) o"
                        ),
                    )
                else:
                    jj = FP16_DEGREES.index(j)
                    c_sb = cpool.tile([P, NIC, OT], f16, name=f"c_{ot}_{j}", tag="c_sb")
                    nc.sync.dma_start(
                        out=c_sb,
                        in_=cj16_ext[jj, :, ot * OT : (ot + 1) * OT].rearrange(
                            "(ic p) o -> p ic o", p=P
                        ),
                    )
                for bt in range(NBT):
                    if j in FP8_DEGREES:
                        for dc in range(NDC):
                            nc.tensor.matmul(
                                psums[bt],
                                lhsT=T8Q[:, j - 1, bt, 2 * dc : 2 * dc + 2, :],
                                rhs=c8_sb[:, 2 * dc : 2 * dc + 2, :],
                                start=False, stop=(closing and dc == NDC - 1),
                                perf_mode=mybir.MatmulPerfMode.DoubleRow,
                            )
                    else:
                        for ic in range(NIC):
                            nc.tensor.matmul(
                                psums[bt], lhsT=T16[:, SLOT[j], bt, ic, :],
                                rhs=c_sb[:, ic, :],
                                start=False, stop=(closing and ic == NIC - 1),
                            )
                    if closing:
                        # bt-major close: copy+store overlaps the remaining MMs
                        o_sb = opool.tile([P, OT], f32)
                        nc.scalar.activation(
                            out=o_sb, in_=psums[bt],
                            func=mybir.ActivationFunctionType.Identity,
                            bias=zbias, scale=OUT_SCALE,
                        )
                        nc.sync.dma_start(
                            out=y_ext[bt * P : (bt + 1) * P, ot * OT : (ot + 1) * OT],
                            in_=o_sb,
                        )

            for ot in range(NOT):
                order = (
                    FP8_DEGREES + FP16_DEGREES
                    if ot == 0
                    else tuple(reversed(FP8_DEGREES + FP16_DEGREES))
                )
                psums = [
                    pspool.tile([P, OT], f32, name=f"ps{ot}_{bt}", tag="psacc")
                    for bt in range(NBT)
                ]
                # HAM warm-up: throwaway rank-1 matmuls fill the PE's
                # dependency-idle window at each pass start so the clock gate
                # opens before the real matmuls. psums[-1]'s real seed
                # (start=True) resets the bank afterwards.
                for _ in range(20 if ot == 0 else 6):
                    nc.tensor.matmul(
                        psums[-1], lhsT=ones_row, rhs=bias_sb[0:1, 0:OT],
                        start=True, stop=True,
                    )
                # seed accumulators with the precomputed bias via K=1 matmul
                for bt in range(NBT):
                    nc.tensor.matmul(
                        psums[bt], lhsT=ones_row,
                        rhs=bias_sb[0:1, ot * OT : (ot + 1) * OT],
                        start=True, stop=False,
                    )
                for j in order:
                    emit_degree(j, ot, psums, closing=(j == order[-1]))

    nc.compile()
    _PROGRAM_CACHE[(repeat, ndev, variant)] = nc
    return nc


def _fp8_neighbor_toward(f8arr, direction):
    """Next representable e4m3 value moving in `direction` (+1/-1 array)."""
    bits = f8arr.view(np.uint8)
    neg = (bits & 0x80) != 0
    mag = (bits & 0x7F).astype(np.int16)
    inc = np.where(neg, -direction, direction)
    mag2 = np.clip(mag + inc, 0, 0x7E)
    bits2 = mag2.astype(np.uint8) | np.where(neg, np.uint8(0x80), np.uint8(0))
    return bits2.view(ml_dtypes.float8_e4m3)


def _zerosum_quant(V):
    """e4m3-quantize V [I, O] with near-zero per-column (axis 0) error sums.

    RTN first, then flip the roundings closest to the midpoint (cheapest in
    added noise) until each column's signed error sum crosses zero.
    """
    e4 = ml_dtypes.float8_e4m3
    V = V.astype(np.float64)
    f8 = V.astype(np.float32).astype(e4)
    Q = f8.astype(np.float64)
    err = Q - V
    D = err.sum(0)
    sgnD = np.sign(D)[None, :]
    alt8 = _fp8_neighbor_toward(f8, (-sgnD).astype(np.int16))
    delta = alt8.astype(np.float64) - Q  # change to column sum if flipped
    eligible = (err * sgnD) > 0
    cost_rank = np.where(eligible, np.abs(err), -1.0)
    order = np.argsort(-cost_rank, axis=0)
    delta_sorted = np.take_along_axis(np.where(eligible, delta, 0.0), order, axis=0)
    cum = np.cumsum(delta_sorted, axis=0)
    Dk = np.abs(D[None, :] + np.concatenate([np.zeros((1, V.shape[1])), cum], 0))
    kbest = np.argmin(Dk, axis=0)  # flips per column
    ranks = np.empty_like(order)
    np.put_along_axis(ranks, order, np.arange(V.shape[0])[:, None].repeat(V.shape[1], 1), axis=0)
    flip = (ranks < kbest[None, :]) & eligible
    return np.where(flip, alt8, f8)


def host_prep(x, cheby_coeffs):
    """Host-side layout/dtype prep of weights + x row shards."""
    C = np.asarray(cheby_coeffs, dtype=np.float64)  # [I, O, 9]

    # bias vector: degree 0, the centering fold of the fp8 degrees, and the
    # -1 fold for degree 8 (whose lhsT is U8 = T8 + 1)
    bias = C[:, :, 0].sum(0)
    for j in FP8_DEGREES:
        cj = CENTER[j]
        if cj != 0.0:
            bias = bias + np.float64(np.float32(cj)) * C[:, :, j].sum(0)
    bias = bias - C[:, :, 8].sum(0)
    bias16 = (bias[None, :] * B_SCALE).astype(np.float16)

    # fp16 slabs for degrees 5..8 (pre-scaled by B_SCALE like everything else)
    cj16 = np.ascontiguousarray(
        np.transpose(C[:, :, list(FP16_DEGREES)], (2, 0, 1)) * B_SCALE
    ).astype(np.float16)

    # fp8 slabs for degrees 1..4: zero-sum e4m3(C_j * B_SCALE),
    # layout [jj, c, p, s, o] with i = c*256 + s*128 + p
    cj8 = np.empty((len(FP8_DEGREES), P, NDC, 2, O_DIM), dtype=ml_dtypes.float8_e4m3)
    for jj, j in enumerate(FP8_DEGREES):
        sign = -1.0 if j == 3 else 1.0  # device stores M3 = -T3
        q = _zerosum_quant(C[:, :, j] * (B_SCALE * sign))  # [I, O] e4m3
        cj8[jj] = q.reshape(NDC, 2, P, O_DIM).transpose(2, 0, 1, 3)

    x = np.asarray(x, dtype=np.float32).reshape(B_FULL, I_DIM).astype(np.float16)
    in_maps = [
        {
            "xs": x[i * B_SH : (i + 1) * B_SH],
            "cj16": cj16,
            "cj8": cj8,
            "bias": bias16,
        }
        for i in range(NCORES)
    ]
    return in_maps


def kernel(x, cheby_coeffs):
    nc = build_program(1)
    in_maps = host_prep(x, cheby_coeffs)
    res = run_bass_kernel_spmd(nc, in_maps, list(range(NCORES)))
    return np.concatenate([r["y"] for r in res.results], axis=0)
